# revision 36
# baseline (speedup 1.0000x reference)
"""Trainium2 Bass kernel for nn_Bspline_segment_calc.

Math: the reference builds a FIXED uniform extended grid (the `grid` input is
unused): knots g_i = -1.6 + 0.2*i, i = 0..16.  With u = 5*x + 8 (x in [0,1) =>
u in [8,13)), every output row is a shift of the cardinal cubic B-spline
kernel:  out[a, r, n] = M4(u - r),  r = 0..12.  Rows 0..4 are identically zero
(assembled host-side; never touched by the device).

Using the symmetry M4(s) = M4(4-s), with a = |u - (r+2)| (folded distance from
the support center) and z = relu(c*(2-a)) where c^3 = 1/6:

    out = z^3 - 4 * relu(z - c)^3

Edge rows 5 and 12 intersect only one polynomial piece over u in [8,13):
    out_5  = relu(c*(9-u))^3 = cube(relu(c - 5c*x))
    out_12 = relu(c*(u-12))^3 = cube(relu(5c*x - 4c))

Engine split (v = 5c*x computed once per chunk by a stock tensor_scalar):
  - V-path interior rows: z = max(min(c(r-4) - v, c(8-r) + v), 0) via a
    hand-authored 2X_2PORT custom-DVE uop (two 4-op chains in the 8 ALU
    blocks -> 2 elem/cycle, f32); then the 8-stage 1x cube-diff op
    out = z^3 - 4*relu(z-c)^3 writes fp16.
  - Edge rows 5/12: cube(max(+-v + C, 0)) as 4-op 2X_2PORT uops, fp16 out.
  - Scalar-path interior rows: ABS + RELU activations produce z; cube-diff
    on the DVE as above.  V/S row assignment balances the two engines.
The 2x programs mirror the stock TENSOR_SCALAR/COPY 2X_2PORT table entries;
the CUSTOM_DVE_ANT instruction's perf_max field (byte-36[7:6]) is set to 2
so the engine dispatches the mode-2 table slot (f32 single-src SBUF
even-dim operands auto-detect 2X_2PORT).

I/O: x is loaded fp16 (quantization adds ~6e-4 L2 rel err), outputs are
written fp16 (~2e-4), both well under the 2e-2 gate and each halving DMA
bytes (in 0.63 MB + out 5 MB per core at ~358 GB/s/core).

Layout: each core's [5, 62500] shard is flattened and padded to 128x2442
(pad value 10.0 maps to basis == 0; the padding cols of row 12's one-piece
formula are wrong there and trimmed host-side).  2 chunks (384 + 2058):
the small first chunk starts compute early (its DMA is issued from the
scalar HWDGE queue, which exits the engine preamble first); chunk 0's 8
output rows go out as ONE chunk-major DMA (contiguous per partition = 128
descriptors); the last chunk's rows DMA per-row as each completes.

Sharding: x is split along N across the 8 cores; each core computes its 8
nonzero basis rows; host assembles the full [5, 13, 500000] output.
"""

import numpy as np

import concourse.bass as bass
import concourse.bacc as bacc
import concourse.tile as tile
from concourse import mybir
from concourse.bass_utils import run_bass_kernel_spmd
import concourse.dve_ops as dve_ops_mod
from concourse.dve_spec import (
    Spec, Src0, C0, C1, C2, Zero, One, relu, sq, maxx, minn, lower, _has_src1,
)
from concourse.dve_uop import (
    DveOpSpec, UopConfig, AluOp as UAluOp, AluInp, DelayInp,
    InpSel, OutPath, OutSel, Trigger, ENABLE,
)

N_CORES = 8
N_ROWS = 5          # x rows
N_BASIS = 13        # output basis rows (rows 0..4 are zero)
R_LO = 5            # first nonzero basis row
N_NZ = N_BASIS - R_LO                # 8 nonzero rows
N_FULL = 500000
N_SHARD = N_FULL // N_CORES          # 62500
N_ELEM = N_ROWS * N_SHARD            # 312500 elements per core
P = 128                              # SBUF partitions (all 16 DMA engines)
FD = -(-N_ELEM // P)                 # 2442 elements per partition
N_PAD = P * FD                       # 312576
X_PAD_VAL = np.float32(10.0)         # maps to u far outside every support
C1V = float(np.float64(6.0) ** (-1.0 / 3.0))   # c with c^3 = 1/6
OUT_DT_NP = np.float16              # device output dtype (halves out DMA)
X_DT_NP = np.float16                # device input dtype (halves in DMA)
BATCH_OUT = True    # chunk-major DRAM layout; one out-DMA per chunk
N_CHUNKS = 2
FIRST_CHUNK = 320   # small first chunk => compute starts sooner
LAST_CHUNK = 0      # 0 = even; else size of the final chunk (small => early exit)
SPLIT_X0 = False    # split first x chunk across sync+scalar queues
X0_ON_SCALAR = True # issue first x chunk from the scalar HWDGE queue
EDGE_ON_V = True    # edge rows fully on DVE (frees 2 ScalarE acts/chunk)
Z_IN_PSUM = False   # route a/z intermediates through PSUM (ScalarE is faster there)
WBUFS = 6
OBUFS = 2
ENABLE_ASSERTS = False
SKIP_INIT_BARRIER = True
# V-independent rows first so VectorE starts without waiting on ScalarE.
ROW_ORDER = [5, 12, 6, 7, 8, 9, 10, 11]
# Order for the final chunk: scalar-path rows first so their (large) output
# DMAs issue early; end on a short edge op to shrink the drain tail.
ROW_ORDER_LAST = None


def _chunks():
    lo, hi, n = 0, FD, N_CHUNKS
    bounds = [0]
    if FIRST_CHUNK and n > 1:
        bounds.append(FIRST_CHUNK)
        lo, n = FIRST_CHUNK, n - 1
    last = LAST_CHUNK if (LAST_CHUNK and n > 1) else 0
    mid_hi, mid_n = hi - last, n - (1 if last else 0)
    bounds += [
        lo + 2 * round(i * (mid_hi - lo) / mid_n / 2) for i in range(1, mid_n)
    ] + [mid_hi]
    if last:
        bounds.append(hi)
    return list(zip(bounds[:-1], bounds[1:]))

# Interior rows computing z on the DVE (rest use ScalarE): engine balance.
V_PATH_RS = (6, 7)
# Extra (row, chunk) pairs on the DVE z-path: fractional S<->V rebalance.
V_PATH_EXTRA = ()
Z_2X = True         # V-path z via hand-authored 2x_2p custom uop (from v)
V_ON_GPSIMD = False # compute v = 5c*x on the (otherwise idle) GpSimd engine
V_ON_SCALAR = False # compute v on ScalarE (Copy act) -- frees the DVE
SPLIT_LAST_ROW = True  # split the final row's op+DMA in half (shorter tail)
SPLIT_FRAC = 0.67   # first-piece fraction of the final-row split
EDGE_2X = True      # edge rows via 2x_2p custom uop (from v, fp16 out)
E12_4X = False      # row 12 via ScalarE relu (fp16) + 4-elem/cycle DVE cube


def _mk_2x2p_uop(kind):
    """Two 4-op chains (elements A/B) in the 8 ALU blocks; f32 2-port mode.

    kind='z':    out = max(min(C0 - v, C1 + v), 0)
    kind='e5':   out = cube(max(C0 - v, 0))
    kind='e12':  out = cube(max(v + C0, 0))
    Conventions mirror the stock TENSOR_SCALAR/COPY 2X_2PORT programs:
    lane k+1 feeds block-0 delay chain k; a chain re-loaded mid-pipe with
    DelayInp.PREV_ALU_OUT captures the previous block's ALU result.
    """
    u = UopConfig()
    u.enable_input(InpSel.SRC_0, 0)      # v_A -> blk0 ALU
    u.enable_input(InpSel.CONST_0, 1)    # -> d0
    u.enable_input(InpSel.CONST_1, 2)    # -> d1
    u.enable_input(InpSel.ZERO, 3)       # -> d2
    u.enable_input(InpSel.SRC_1, 4)      # v_B -> d3
    u.enable_input(InpSel.SRC_0, 5)      # v_A again -> d4 (z only)
    u.require_inp0 = ENABLE
    u.require_inp1 = ENABLE
    u.trigger = (Trigger.SRC_TENSOR_DONE, Trigger.NONE, Trigger.NONE)
    b = u.datapath_config
    if kind == "z":
        # chain: p=C0-v; q=C1+v; m=min(q,p); z=max(m,0)
        b[0].enable_alu(UAluOp.SUBTRACT, AluInp.PREV_DELAY_0, AluInp.PREV_ALU_OUT)
        b[0].pass_through_delay(0, 1, 2, 3, 4)
        b[1].enable_alu(UAluOp.ADD, AluInp.PREV_DELAY_1, AluInp.PREV_DELAY_4)
        b[1].pass_through_delay(0, 1, 2, 3)
        b[1].enable_delay_from_src(DelayInp.PREV_ALU_OUT, 5)      # p_A
        b[2].enable_alu(UAluOp.MIN, AluInp.PREV_ALU_OUT, AluInp.PREV_DELAY_5)
        b[2].pass_through_delay(0, 1, 2, 3)
        b[3].enable_alu(UAluOp.MAX, AluInp.PREV_ALU_OUT, AluInp.PREV_DELAY_2)
        b[3].pass_through_delay(0, 1, 2, 3)
        b[4].enable_alu(UAluOp.SUBTRACT, AluInp.PREV_DELAY_0, AluInp.PREV_DELAY_3)
        b[4].pass_through_delay(1, 2, 3)
        b[4].enable_delay_from_src(DelayInp.PREV_ALU_OUT, 4)      # z_A
        b[5].enable_alu(UAluOp.ADD, AluInp.PREV_DELAY_1, AluInp.PREV_DELAY_3)
        b[5].pass_through_delay(2, 4)
        b[5].enable_delay_from_src(DelayInp.PREV_ALU_OUT, 5)      # p_B
        b[6].enable_alu(UAluOp.MIN, AluInp.PREV_ALU_OUT, AluInp.PREV_DELAY_5)
        b[6].pass_through_delay(2, 4)
        b[7].enable_alu(UAluOp.MAX, AluInp.PREV_ALU_OUT, AluInp.PREV_DELAY_2)
        b[7].pass_through_delay(4)
    else:
        # chain: r=max(C0-v,0) or max(v+C0,0); out=r*r*r
        if kind == "e5":
            b[0].enable_alu(UAluOp.SUBTRACT, AluInp.PREV_DELAY_0, AluInp.PREV_ALU_OUT)
        else:
            b[0].enable_alu(UAluOp.ADD, AluInp.PREV_ALU_OUT, AluInp.PREV_DELAY_0)
        b[0].pass_through_delay(0, 2, 3)
        b[1].enable_alu(UAluOp.MAX, AluInp.PREV_ALU_OUT, AluInp.PREV_DELAY_2)
        b[1].pass_through_delay(0, 2, 3)
        b[2].enable_alu(UAluOp.MULTIPLY, AluInp.PREV_ALU_OUT, AluInp.PREV_ALU_OUT)
        b[2].pass_through_delay(0, 2, 3)
        b[2].enable_delay_from_src(DelayInp.PREV_ALU_OUT, 4)      # r_A
        b[3].enable_alu(UAluOp.MULTIPLY, AluInp.PREV_ALU_OUT, AluInp.PREV_DELAY_4)
        b[3].pass_through_delay(0, 2, 3)
        if kind == "e5":
            b[4].enable_alu(UAluOp.SUBTRACT, AluInp.PREV_DELAY_0, AluInp.PREV_DELAY_3)
        else:
            b[4].enable_alu(UAluOp.ADD, AluInp.PREV_DELAY_3, AluInp.PREV_DELAY_0)
        b[4].pass_through_delay(2)
        b[4].enable_delay_from_src(DelayInp.PREV_ALU_OUT, 4)      # out_A
        b[5].enable_alu(UAluOp.MAX, AluInp.PREV_ALU_OUT, AluInp.PREV_DELAY_2)
        b[5].pass_through_delay(4)
        b[6].enable_alu(UAluOp.MULTIPLY, AluInp.PREV_ALU_OUT, AluInp.PREV_ALU_OUT)
        b[6].pass_through_delay(4)
        b[6].enable_delay_from_src(DelayInp.PREV_ALU_OUT, 5)      # r_B
        b[7].enable_alu(UAluOp.MULTIPLY, AluInp.PREV_ALU_OUT, AluInp.PREV_DELAY_5)
        b[7].pass_through_delay(4)
    u.out[OutPath.WR0_LO] = OutSel.DELAY_4
    u.out_enable[OutPath.WR0_LO] = ENABLE
    u.out[OutPath.WR1_LO] = OutSel.ALU_OUT
    u.out_enable[OutPath.WR1_LO] = ENABLE
    return u


def _mk_cube_uop(mode):
    """cube = x*x*x per element, fp16 packed operands.
    mode='m1' (2X_1PORT: elems A,B from SRC_0/SRC_0_HI) or
    mode='m4' (4X_2PORT: elems A..D from SRC_0/_HI/SRC_1/_HI)."""
    u = UopConfig()
    u.enable_input(InpSel.SRC_0, 0)       # A -> blk0 ALU
    u.enable_input(InpSel.SRC_0, 1)       # A -> d0
    u.enable_input(InpSel.SRC_0_HI, 2)    # B -> d1
    u.require_inp0 = ENABLE
    if mode == "m4":
        u.enable_input(InpSel.SRC_1, 3)       # C -> d2
        u.enable_input(InpSel.SRC_1_HI, 4)    # D -> d3
        u.require_inp1 = ENABLE
    u.trigger = (Trigger.SRC_TENSOR_DONE, Trigger.NONE, Trigger.NONE)
    b = u.datapath_config
    b[0].enable_alu(UAluOp.MULTIPLY, AluInp.PREV_ALU_OUT, AluInp.PREV_ALU_OUT)
    b[0].pass_through_delay(*((0, 1, 2, 3) if mode == "m4" else (0, 1)))
    b[1].enable_alu(UAluOp.MULTIPLY, AluInp.PREV_ALU_OUT, AluInp.PREV_DELAY_0)
    b[1].pass_through_delay(*((1, 2, 3) if mode == "m4" else (1,)))
    b[2].enable_alu(UAluOp.MULTIPLY, AluInp.PREV_DELAY_1, AluInp.PREV_DELAY_1)
    b[2].pass_through_delay(*((1, 2, 3) if mode == "m4" else (1,)))
    b[2].enable_delay_from_src(DelayInp.PREV_ALU_OUT, 4)       # A^3
    b[3].enable_alu(UAluOp.MULTIPLY, AluInp.PREV_ALU_OUT, AluInp.PREV_DELAY_1)
    b[3].pass_through_delay(*((2, 3, 4) if mode == "m4" else (4,)))
    if mode == "m1":
        b[4].pass_through_delay(4)
        b[4].enable_delay_from_src(DelayInp.PREV_ALU_OUT, 5)   # B^3
        for k in (5, 6, 7):
            b[k].pass_through_delay(4, 5)
        u.out[OutPath.WR0_LO] = OutSel.DELAY_4
        u.out_enable[OutPath.WR0_LO] = ENABLE
        u.out[OutPath.WR0_HI] = OutSel.DELAY_5
        u.out_enable[OutPath.WR0_HI] = ENABLE
        return u
    b[4].enable_alu(UAluOp.MULTIPLY, AluInp.PREV_DELAY_2, AluInp.PREV_DELAY_2)
    b[4].pass_through_delay(2, 3, 4)
    b[4].enable_delay_from_src(DelayInp.PREV_ALU_OUT, 5)       # B^3
    b[5].enable_alu(UAluOp.MULTIPLY, AluInp.PREV_ALU_OUT, AluInp.PREV_DELAY_2)
    b[5].pass_through_delay(3, 4, 5)
    b[6].enable_alu(UAluOp.MULTIPLY, AluInp.PREV_DELAY_3, AluInp.PREV_DELAY_3)
    b[6].pass_through_delay(3, 4, 5)
    b[6].enable_delay_from_src(DelayInp.PREV_ALU_OUT, 0)       # C^3
    b[7].enable_alu(UAluOp.MULTIPLY, AluInp.PREV_ALU_OUT, AluInp.PREV_DELAY_3)
    b[7].pass_through_delay(0, 4, 5)
    u.out[OutPath.WR0_LO] = OutSel.DELAY_4
    u.out_enable[OutPath.WR0_LO] = ENABLE
    u.out[OutPath.WR0_HI] = OutSel.DELAY_5
    u.out_enable[OutPath.WR0_HI] = ENABLE
    u.out[OutPath.WR1_LO] = OutSel.DELAY_0
    u.out_enable[OutPath.WR1_LO] = ENABLE
    u.out[OutPath.WR1_HI] = OutSel.ALU_OUT
    u.out_enable[OutPath.WR1_HI] = ENABLE
    return u


class _PerfDveOp:
    """Duck-typed DveOp whose compiled DveOpSpec carries hand-authored
    perf-mode programs (2X_1PORT filler + 2X_2PORT) with perf_max=2."""

    def __init__(self, name, spec, uop_kind):
        self.name = name
        self.spec = spec
        self.subdim = False
        self.uops_sha = {}
        self._uop_kind = uop_kind
        self._cache = {}

    def compile(self, ver):
        if ver in self._cache:
            return self._cache[ver]
        s = DveOpSpec(
            name=self.name,
            opcode=dve_ops_mod.get_dve_sub_opcode(self.name),
            uops=lower(self.spec, ver=ver),
            rd1_en=_has_src1(self.spec),
        )
        if ver == "v3":
            assert len(s.uops) == 1, (self.name, len(s.uops))
            if self._uop_kind == "cube4x":
                m1 = _mk_cube_uop("m1")
                s.uops_2x = [m1]
                s.uops_2x_2p = [m1]   # unreachable (16-bit srcs go m1/m4)
                s.uops_4x = [_mk_cube_uop("m4")]
                s.perf_max = 3
            else:
                pu = _mk_2x2p_uop(self._uop_kind)
                s.uops_2x = [pu]  # unreachable filler (f32 srcs never pack)
                s.uops_2x_2p = [pu]
                s.perf_max = 2
        self._cache[ver] = s
        return s


def _register_dve_op(name, spec):
    for op in dve_ops_mod.OPS:
        if op.name == name:
            return op
    opcode = dve_ops_mod._CUSTOM_DVE_ROW_BASE + len(dve_ops_mod.OPS)
    assert opcode < 0x20, "custom DVE row overflow"
    shas = {}
    for ver in ("v3", "v4"):
        uops = lower(spec, ver=ver)
        shas[ver] = DveOpSpec(
            name=name, opcode=opcode, uops=uops, rd1_en=_has_src1(spec)
        ).sha(ver)
    op = dve_ops_mod.DveOp(name, spec, subdim=False, uops_sha=shas)
    dve_ops_mod.OPS.append(op)
    dve_ops_mod._SUB_OPCODE_FOR_NAME[name] = opcode
    dve_ops_mod.CUSTOM_DVE_SPECS[name] = spec
    return op


def _register_perf_dve_op(name, spec, uop_kind):
    for op in dve_ops_mod.OPS:
        if op.name == name:
            return op
    opcode = dve_ops_mod._CUSTOM_DVE_ROW_BASE + len(dve_ops_mod.OPS)
    assert opcode < 0x20, "custom DVE row overflow"
    op = _PerfDveOp(name, spec, uop_kind)
    dve_ops_mod.OPS.append(op)
    dve_ops_mod._SUB_OPCODE_FOR_NAME[name] = opcode
    dve_ops_mod.CUSTOM_DVE_SPECS[name] = spec
    op.compile("v3")  # validate lowering + perf programs early
    return op


def _get_z2x_op():
    # out = max(min(s0 - v, s1 + v), 0)   (4 ALU ops; 2 elem/cycle at 2x_2p)
    spec = Spec(
        body=maxx(minn(C0 - Src0, C1 + Src0), Zero),
        reference=lambda in0, in1, s0, s1, imm2: np.maximum(
            np.minimum(s0 - in0, s1 + in0), np.float32(0.0)
        ).astype(np.float32),
    )
    return _register_perf_dve_op("BSPLINE_Z2X_ANT", spec, "z")


def _get_edge5_2x_op():
    # out = cube(max(s0 - v, 0))
    r = maxx(C0 - Src0, Zero)
    spec = Spec(
        body=sq(r) * r,
        reference=lambda in0, in1, s0, s1, imm2: (
            np.maximum(s0 - in0, np.float32(0.0)).astype(np.float32) ** 3
        ).astype(np.float32),
    )
    return _register_perf_dve_op("BSPLINE_E5_2X_ANT", spec, "e5")


def _get_edge12_2x_op():
    # out = cube(max(v + s0, 0))
    r = maxx(Src0 + C0, Zero)
    spec = Spec(
        body=sq(r) * r,
        reference=lambda in0, in1, s0, s1, imm2: (
            np.maximum(in0 + s0, np.float32(0.0)).astype(np.float32) ** 3
        ).astype(np.float32),
    )
    return _register_perf_dve_op("BSPLINE_E12_2X_ANT", spec, "e12")


def _emit_perf_dve(nc, op, *, out, in0, s0=0.0, s1=0.0, perf_max=2):
    bi = nc.vector._custom_dve(op, out=out, in0=in0, s0=s0, s1=s1)
    bi.ins.perf_max = perf_max
    return bi


def _get_cube4x_op():
    # out = in0^3, fp16 packed: 2 elem/cyc at 2X_1PORT, 4 at 4X_2PORT
    spec = Spec(
        body=sq(Src0) * Src0,
        reference=lambda in0, in1, s0, s1, imm2: (
            in0.astype(np.float32) ** 3
        ).astype(np.float32),
    )
    return _register_perf_dve_op("BSPLINE_CUBE4X_ANT", spec, "cube4x")


def _get_cube_diff_op():
    # out = in0^3 - imm2 * relu(in0 - s0)^3        (8 ALU stages)
    r = relu(Src0 - C0)
    body = sq(Src0) * Src0 - sq(r) * r * C2
    spec = Spec(
        body=body,
        reference=lambda in0, in1, s0, s1, imm2: (
            in0.astype(np.float32) ** 3
            - np.maximum(in0 - s0, np.float32(0.0)).astype(np.float32) ** 3 * imm2
        ).astype(np.float32),
    )
    return _register_dve_op("BSPLINE_CUBE_DIFF_ANT", spec)


def _get_z_op():
    # out = relu((2 - |in0*imm2 + s0|) * s1)       (7 ALU stages)
    w = Src0 * C2 + C0
    a = maxx(w, Zero - w)
    body = relu(((One + One) - a) * C1)
    spec = Spec(
        body=body,
        reference=lambda in0, in1, s0, s1, imm2: np.maximum(
            (np.float32(2.0) - np.abs(in0 * imm2 + s0)) * s1, np.float32(0.0)
        ).astype(np.float32),
    )
    return _register_dve_op("BSPLINE_Z_ANT", spec)


def _get_cube_op():
    # out = in0^3                                  (2 ALU stages)
    spec = Spec(
        body=sq(Src0) * Src0,
        reference=lambda in0, in1, s0, s1, imm2: (
            in0.astype(np.float32) ** 3
        ).astype(np.float32),
    )
    return _register_dve_op("BSPLINE_CUBE_ANT", spec)


def _get_edge_cube_op():
    # out = relu(in0*s0 + s1)^3                    (5 ALU stages)
    r = relu(Src0 * C0 + C1)
    spec = Spec(
        body=sq(r) * r,
        reference=lambda in0, in1, s0, s1, imm2: (
            np.maximum(in0 * s0 + s1, np.float32(0.0)).astype(np.float32) ** 3
        ).astype(np.float32),
    )
    return _register_dve_op("BSPLINE_EDGE_CUBE_ANT", spec)


def _register_const(nc, value):
    """Make `value` usable as an activation bias (const_aps lookup).
    Must be called inside the TileContext: the memset is tracked by Tile."""
    f32 = mybir.dt.float32
    key = (f32, float(value))
    if key in nc.const_aps.aps:
        return
    t = nc.alloc_sbuf_tensor(f"const-f32-{float(value)}", [128, 1], f32)
    nc.vector.memset(t.ap(), float(value))
    nc.const_aps.aps[key] = t.ap()


def _build_bass():
    cube_diff_op = _get_cube_diff_op()
    z_op = _get_z_op()
    cube_op = _get_cube_op()
    edge_cube_op = _get_edge_cube_op()
    z2x_op = _get_z2x_op()
    edge5_2x_op = _get_edge5_2x_op()
    edge12_2x_op = _get_edge12_2x_op()
    cube4x_op = _get_cube4x_op()
    f32 = mybir.dt.float32
    # Skip Bass.__init__'s trailing all-engine barrier (only guards its
    # 0.0/1.0 const memsets; the earlier _nrt_pseudo_barrier already orders
    # the semaphore clears).  The only in-kernel reader of those consts is
    # the throwaway table-warm activation below.  Saves ~2us of preamble.
    if SKIP_INIT_BARRIER:
        _orig_barrier = bass.Bass.all_engine_barrier
        bass.Bass.all_engine_barrier = lambda self: None
        try:
            nc = bacc.Bacc(
                "TRN2", target_bir_lowering=False, debug=False,
                num_devices=N_CORES, enable_asserts=ENABLE_ASSERTS,
            )
        finally:
            bass.Bass.all_engine_barrier = _orig_barrier
    else:
        nc = bacc.Bacc(
            "TRN2", target_bir_lowering=False, debug=False,
            num_devices=N_CORES, enable_asserts=ENABLE_ASSERTS,
        )
    f16 = mybir.dt.float16
    x_dt = {np.float16: f16, np.float32: f32}[X_DT_NP]
    x_dram = nc.dram_tensor("x", [N_PAD], x_dt, kind="ExternalInput")
    if BATCH_OUT:
        # chunk-major layout: partition p's row-r chunk-c data lives at
        # [p, N_NZ*lo_c + (r-R_LO)*ch_c + f] -- each chunk's 8 rows are one
        # contiguous run per partition, so one DMA = 128 descriptors.
        out_dram = nc.dram_tensor("out", [P, N_NZ * FD], f16, kind="ExternalOutput")
    else:
        out_dram = nc.dram_tensor("out", [N_NZ, N_PAD], f16, kind="ExternalOutput")
    xv = x_dram.ap().rearrange("(p f) -> p f", p=P)

    with tile.TileContext(nc) as tc:
        with (
            tc.tile_pool(name="const", bufs=1) as cpool,
            tc.tile_pool(name="work", bufs=WBUFS) as wpool,
            tc.tile_pool(name="obuf", bufs=OBUFS) as opool,
            tc.tile_pool(name="psum", bufs=2, space="PSUM") as ppool,
        ):
            zpool = ppool if Z_IN_PSUM else wpool
            x_tile = cpool.tile([P, FD], x_dt, tag="x")
            for ci, (lo, hi) in enumerate(_chunks()):
                if ci == 0 and SPLIT_X0:
                    # halve the first chunk across both HWDGE queues so
                    # compute starts sooner
                    nc.sync.dma_start(out=x_tile[:64, lo:hi], in_=xv[:64, lo:hi])
                    nc.scalar.dma_start(out=x_tile[64:, lo:hi], in_=xv[64:, lo:hi])
                elif ci == 0 and X0_ON_SCALAR:
                    # scalar queue is free ~1us before sync (which is still in
                    # its preamble DRAIN); emitting chunk0 there (before any
                    # ACTIVATE, so ahead of the ACT_TABLE_LOAD) starts the
                    # input pipeline sooner
                    nc.scalar.dma_start(out=x_tile[:, lo:hi], in_=xv[:, lo:hi])
                else:
                    nc.sync.dma_start(out=x_tile[:, lo:hi], in_=xv[:, lo:hi])

            warm = cpool.tile([P, 1], f32, tag="warm")
            nc.scalar.activation(
                warm[:], nc.const_aps.aps[(f32, 0.0)][:P, :],
                mybir.ActivationFunctionType.Abs, bias=0.0, scale=1.0,
            )
            for r in range(R_LO + 1, N_BASIS - 1):
                if r not in V_PATH_RS:
                    _register_const(nc, float(6 - r))
            _register_const(nc, 2.0 * C1V)
            _register_const(nc, C1V)          # bias for edge row 5
            _register_const(nc, -4.0 * C1V)   # bias for edge row 12

            chunks = _chunks()
            for ci, (lo, hi) in enumerate(chunks):
                rows = list(
                    (ROW_ORDER_LAST if ci == len(chunks) - 1 and ROW_ORDER_LAST
                     else ROW_ORDER) or range(R_LO, N_BASIS)
                )
                ch = hi - lo
                xs = x_tile[:, lo:hi]
                batch = BATCH_OUT and ci < len(chunks) - 1
                if batch:
                    o8_t = opool.tile(
                        [P, N_NZ * ch], f16, tag="o8", name=f"o8_{ci}"
                    )
                else:
                    o8_t = None
                need_v = (Z_2X and any(
                    r2 in V_PATH_RS or (r2, ci) in V_PATH_EXTRA
                    for r2 in range(R_LO + 1, N_BASIS - 1)
                )) or EDGE_2X
                if need_v:
                    # v = 5c*x, shared by the 2x z/edge ops of this chunk
                    v_t = wpool.tile([P, ch], f32, tag="v")
                    if V_ON_SCALAR:
                        nc.scalar.activation(
                            v_t[:], xs, mybir.ActivationFunctionType.Copy,
                            bias=0.0, scale=5.0 * C1V,
                        )
                    else:
                        eng = nc.gpsimd if V_ON_GPSIMD else nc.vector
                        eng.tensor_scalar_mul(v_t[:], xs, 5.0 * C1V)
                for r in rows:
                    on_v = r in V_PATH_RS or (r, ci) in V_PATH_EXTRA
                    if batch:
                        ri = r - R_LO
                        o_ap = o8_t[:, ri * ch : (ri + 1) * ch]
                    else:
                        o_t = wpool.tile([P, ch], f16, tag="o")
                        o_ap = o_t[:]
                    if r == R_LO and EDGE_2X:
                        # out_5 = cube(max(c - v, 0))  -- 2 elem/cycle
                        _emit_perf_dve(
                            nc, edge5_2x_op, out=o_ap, in0=v_t[:], s0=C1V,
                        )
                    elif r == N_BASIS - 1 and E12_4X:
                        # out_12: ScalarE relu (fp16) + 4-elem/cycle cube
                        et = wpool.tile([P, ch], f16, tag="a")
                        nc.scalar.activation(
                            et[:], xs, mybir.ActivationFunctionType.Relu,
                            bias=-4.0 * C1V, scale=5.0 * C1V,
                        )
                        _emit_perf_dve(
                            nc, cube4x_op, out=o_ap, in0=et[:], perf_max=3,
                        )
                    elif r == N_BASIS - 1 and EDGE_2X:
                        # out_12 = cube(max(v - 4c, 0))  -- 2 elem/cycle
                        _emit_perf_dve(
                            nc, edge12_2x_op, out=o_ap, in0=v_t[:],
                            s0=-4.0 * C1V,
                        )
                    elif r == R_LO and EDGE_ON_V:
                        # out_5 = cube(relu(-5c*x + c))  -- one DVE op
                        nc.vector._custom_dve(
                            edge_cube_op, out=o_ap, in0=xs,
                            s0=-5.0 * C1V, s1=C1V,
                        )
                    elif r == N_BASIS - 1 and EDGE_ON_V:
                        # out_12 = cube(relu(5c*x - 4c))  -- one DVE op
                        nc.vector._custom_dve(
                            edge_cube_op, out=o_ap, in0=xs,
                            s0=5.0 * C1V, s1=-4.0 * C1V,
                        )
                    elif r == R_LO:
                        # out_5 = cube(relu(c*(1 - 5x)))
                        z_t = wpool.tile([P, hi - lo], f32, tag="z")
                        nc.scalar.activation(
                            z_t[:], xs, mybir.ActivationFunctionType.Relu,
                            bias=C1V, scale=-5.0 * C1V,
                        )
                        nc.vector._custom_dve(cube_op, out=o_ap, in0=z_t[:])
                    elif r == N_BASIS - 1:
                        # out_12 = cube(relu(c*(5x - 4)))
                        z_t = wpool.tile([P, hi - lo], f32, tag="z")
                        nc.scalar.activation(
                            z_t[:], xs, mybir.ActivationFunctionType.Relu,
                            bias=-4.0 * C1V, scale=5.0 * C1V,
                        )
                        nc.vector._custom_dve(cube_op, out=o_ap, in0=z_t[:])
                    else:
                        z_t = (wpool if on_v else zpool).tile(
                            [P, hi - lo], f32, tag="z"
                        )
                        if on_v and Z_2X:
                            # z = max(min(c(r-4) - v, c(8-r) + v), 0)
                            _emit_perf_dve(
                                nc, z2x_op, out=z_t[:], in0=v_t[:],
                                s0=(r - 4) * C1V, s1=(8 - r) * C1V,
                            )
                        elif on_v:
                            # z = relu((2 - |5x + (6-r)|) * c)   -- one DVE op
                            nc.vector._custom_dve(
                                z_op, out=z_t[:], in0=xs,
                                s0=float(6 - r), s1=C1V, imm2=5.0,
                            )
                        else:
                            # a = |5x + (6-r)|; z = relu(-c*a + 2c) -- ScalarE
                            a_t = wpool.tile([P, hi - lo], f32, tag="a")
                            nc.scalar.activation(
                                a_t[:], xs, mybir.ActivationFunctionType.Abs,
                                bias=float(6 - r), scale=5.0,
                            )
                            nc.scalar.activation(
                                z_t[:], a_t[:],
                                mybir.ActivationFunctionType.Relu,
                                bias=2.0 * C1V, scale=-C1V,
                            )
                        split = (
                            SPLIT_LAST_ROW and not batch and r == rows[-1]
                            and BATCH_OUT
                        )
                        if split:
                            # final row: two half-width ops + DMAs so the
                            # first half's writeback overlaps the second
                            # half's compute (shorter drain tail)
                            h = int(ch * SPLIT_FRAC) & ~1
                            base = N_NZ * lo + (r - R_LO) * ch
                            for a, b2 in ((0, h), (h, ch)):
                                nc.vector._custom_dve(
                                    cube_diff_op, out=o_ap[:, a:b2],
                                    in0=z_t[:, a:b2], s0=C1V, imm2=4.0,
                                )
                                nc.sync.dma_start(
                                    out=out_dram.ap()[:, base + a : base + b2],
                                    in_=o_ap[:, a:b2],
                                )
                        else:
                            # out = z^3 - 4*relu(z - c)^3
                            nc.vector._custom_dve(
                                cube_diff_op, out=o_ap, in0=z_t[:],
                                s0=C1V, imm2=4.0,
                            )
                    if r in (R_LO, N_BASIS - 1):
                        split = False
                    if not batch and not split:
                        if BATCH_OUT:
                            ov = out_dram.ap()[
                                :, N_NZ * lo + (r - R_LO) * ch :
                                N_NZ * lo + (r - R_LO + 1) * ch
                            ]
                        else:
                            ov = out_dram.ap()[r - R_LO, :].rearrange(
                                "(p f) -> p f", p=P
                            )[:, lo:hi]
                        nc.sync.dma_start(out=ov, in_=o_ap)
                if batch:
                    ov = out_dram.ap()[:, N_NZ * lo : N_NZ * hi]
                    nc.sync.dma_start(out=ov, in_=o8_t[:])
    nc.compile()
    return nc


def make_shard(x, i):
    """Build core i's padded input shard in the device input dtype."""
    sh = np.full(N_PAD, X_PAD_VAL, dtype=X_DT_NP)
    sh[:N_ELEM] = np.ascontiguousarray(
        x[:, i * N_SHARD : (i + 1) * N_SHARD]
    ).reshape(-1).astype(X_DT_NP)
    return sh


_NC_CACHE = None


def _get_nc():
    global _NC_CACHE
    if _NC_CACHE is None:
        _NC_CACHE = _build_bass()
    return _NC_CACHE


def kernel(x, grid=None, k=None, **_ignored):
    x = np.asarray(x, dtype=np.float32)
    assert x.shape == (N_ROWS, N_FULL), x.shape
    nc = _get_nc()
    in_maps = [{"x": make_shard(x, i)} for i in range(N_CORES)]
    res = run_bass_kernel_spmd(nc, in_maps, list(range(N_CORES))).results
    full = np.zeros((N_ROWS, N_BASIS, N_FULL), dtype=np.float32)
    for i in range(N_CORES):
        o = np.asarray(res[i]["out"])
        if BATCH_OUT:
            # [P, N_NZ*FD] chunk-major -> [N_NZ, P, FD] -> [N_NZ, N_PAD]
            rows = np.empty((N_NZ, P, FD), dtype=np.float32)
            for lo, hi in _chunks():
                seg = o[:, N_NZ * lo : N_NZ * hi].reshape(P, N_NZ, hi - lo)
                rows[:, :, lo:hi] = seg.transpose(1, 0, 2)
            o = rows.reshape(N_NZ, N_PAD)
        else:
            o = o.astype(np.float32)  # [N_NZ, N_PAD]
        full[:, R_LO:, i * N_SHARD : (i + 1) * N_SHARD] = o[:, :N_ELEM].reshape(
            N_NZ, N_ROWS, N_SHARD
        ).transpose(1, 0, 2)
    return full



# revision 37
# speedup vs baseline: 1.2099x; 1.2099x over previous
"""Trainium2 Bass kernel for nn_Bspline_segment_calc.

Math: the reference builds a FIXED uniform extended grid (the `grid` input is
unused): knots g_i = -1.6 + 0.2*i, i = 0..16.  With u = 5*x + 8 (x in [0,1) =>
u in [8,13)), every output row is a shift of the cardinal cubic B-spline
kernel:  out[a, r, n] = M4(u - r),  r = 0..12.  Rows 0..4 are identically zero
(assembled host-side; never touched by the device).

Using the symmetry M4(s) = M4(4-s), with a = |u - (r+2)| (folded distance from
the support center) and z = relu(c*(2-a)) where c^3 = 1/6:

    out = z^3 - 4 * relu(z - c)^3

Edge rows 5 and 12 intersect only one polynomial piece over u in [8,13):
    out_5  = relu(c*(9-u))^3 = cube(relu(c - 5c*x))
    out_12 = relu(c*(u-12))^3 = cube(relu(5c*x - 4c))

Engine split (v = 5c*x computed once per chunk by a stock tensor_scalar):
  - V-path interior rows: z = max(min(c(r-4) - v, c(8-r) + v), 0) via a
    hand-authored 2X_2PORT custom-DVE uop (two 4-op chains in the 8 ALU
    blocks -> 2 elem/cycle, f32); then the 8-stage 1x cube-diff op
    out = z^3 - 4*relu(z-c)^3 writes fp16.
  - Edge rows 5/12: cube(max(+-v + C, 0)) as 4-op 2X_2PORT uops, fp16 out.
  - Scalar-path interior rows: ABS + RELU activations produce z; cube-diff
    on the DVE as above.  V/S row assignment balances the two engines.
The 2x programs mirror the stock TENSOR_SCALAR/COPY 2X_2PORT table entries;
the CUSTOM_DVE_ANT instruction's perf_max field (byte-36[7:6]) is set to 2
so the engine dispatches the mode-2 table slot (f32 single-src SBUF
even-dim operands auto-detect 2X_2PORT).

I/O: x is loaded fp16 (quantization adds ~6e-4 L2 rel err), outputs are
written fp16 (~2e-4), both well under the 2e-2 gate and each halving DMA
bytes (in 0.63 MB + out 5 MB per core at ~358 GB/s/core).

Layout: each core's [5, 62500] shard is flattened and padded to 128x2442
(pad value 10.0 maps to basis == 0; the padding cols of row 12's one-piece
formula are wrong there and trimmed host-side).  2 chunks (384 + 2058):
the small first chunk starts compute early (its DMA is issued from the
scalar HWDGE queue, which exits the engine preamble first); chunk 0's 8
output rows go out as ONE chunk-major DMA (contiguous per partition = 128
descriptors); the last chunk's rows DMA per-row as each completes.

Sharding: x is split along N across the 8 cores; each core computes its 8
nonzero basis rows; host assembles the full [5, 13, 500000] output.
"""

import numpy as np

import concourse.bass as bass
import concourse.bacc as bacc
import concourse.tile as tile
from concourse import mybir
from concourse.bass_utils import run_bass_kernel_spmd
import concourse.dve_ops as dve_ops_mod
from concourse.dve_spec import (
    Spec, Src0, C0, C1, C2, Zero, One, relu, sq, maxx, minn, lower, _has_src1,
)
from concourse.dve_uop import (
    DveOpSpec, UopConfig, AluOp as UAluOp, AluInp, DelayInp,
    InpSel, OutPath, OutSel, Trigger, ENABLE,
)

N_CORES = 8
N_ROWS = 5          # x rows
N_BASIS = 13        # output basis rows (rows 0..4 are zero)
R_LO = 5            # first nonzero basis row
N_NZ = N_BASIS - R_LO                # 8 nonzero rows
N_FULL = 500000
N_SHARD = N_FULL // N_CORES          # 62500
N_ELEM = N_ROWS * N_SHARD            # 312500 elements per core
P = 128                              # SBUF partitions (all 16 DMA engines)
FD = -(-N_ELEM // P)                 # 2442 elements per partition
N_PAD = P * FD                       # 312576
X_PAD_VAL = np.float32(10.0)         # maps to u far outside every support
C1V = float(np.float64(6.0) ** (-1.0 / 3.0))   # c with c^3 = 1/6
OUT_DT_NP = np.float16              # device output dtype (halves out DMA)
X_DT_NP = np.float16                # device input dtype (halves in DMA)
BATCH_OUT = True    # chunk-major DRAM layout; one out-DMA per chunk
N_CHUNKS = 2
FIRST_CHUNK = 320   # small first chunk => compute starts sooner
LAST_CHUNK = 0      # 0 = even; else size of the final chunk (small => early exit)
SPLIT_X0 = False    # split first x chunk across sync+scalar queues
X0_ON_SCALAR = True # issue first x chunk from the scalar HWDGE queue
EDGE_ON_V = True    # edge rows fully on DVE (frees 2 ScalarE acts/chunk)
Z_IN_PSUM = False   # route a/z intermediates through PSUM (ScalarE is faster there)
WBUFS = 5
OBUFS = 2
ENABLE_ASSERTS = False
SKIP_INIT_BARRIER = True
# V-independent rows first so VectorE starts without waiting on ScalarE.
ROW_ORDER = [5, 12, 6, 7, 8, 9, 10, 11]
# Order for the final chunk: scalar-path rows first so their (large) output
# DMAs issue early; end on a short edge op to shrink the drain tail.
ROW_ORDER_LAST = None


def _chunks():
    lo, hi, n = 0, FD, N_CHUNKS
    bounds = [0]
    if FIRST_CHUNK and n > 1:
        bounds.append(FIRST_CHUNK)
        lo, n = FIRST_CHUNK, n - 1
    last = LAST_CHUNK if (LAST_CHUNK and n > 1) else 0
    mid_hi, mid_n = hi - last, n - (1 if last else 0)
    bounds += [
        lo + 2 * round(i * (mid_hi - lo) / mid_n / 2) for i in range(1, mid_n)
    ] + [mid_hi]
    if last:
        bounds.append(hi)
    return list(zip(bounds[:-1], bounds[1:]))

# Interior rows computing z on the DVE (rest use ScalarE): engine balance.
V_PATH_RS = (6, 7)
# Extra (row, chunk) pairs on the DVE z-path: fractional S<->V rebalance.
V_PATH_EXTRA = ()
Z_2X = True         # V-path z via hand-authored 2x_2p custom uop (from v)
V_ON_GPSIMD = False # compute v = 5c*x on the (otherwise idle) GpSimd engine
V_ON_SCALAR = False # compute v on ScalarE (Copy act) -- frees the DVE
SPLIT_LAST_ROW = True  # split the final row's op+DMA in half (shorter tail)
SPLIT_FRAC = 0.67   # first-piece fraction of the final-row split
EDGE_2X = True      # edge rows via 2x_2p custom uop (from v, fp16 out)
E12_4X = False      # row 12 via ScalarE relu (fp16) + 4-elem/cycle DVE cube


def _mk_2x2p_uop(kind):
    """Two 4-op chains (elements A/B) in the 8 ALU blocks; f32 2-port mode.

    kind='z':    out = max(min(C0 - v, C1 + v), 0)
    kind='e5':   out = cube(max(C0 - v, 0))
    kind='e12':  out = cube(max(v + C0, 0))
    Conventions mirror the stock TENSOR_SCALAR/COPY 2X_2PORT programs:
    lane k+1 feeds block-0 delay chain k; a chain re-loaded mid-pipe with
    DelayInp.PREV_ALU_OUT captures the previous block's ALU result.
    """
    u = UopConfig()
    u.enable_input(InpSel.SRC_0, 0)      # v_A -> blk0 ALU
    u.enable_input(InpSel.CONST_0, 1)    # -> d0
    u.enable_input(InpSel.CONST_1, 2)    # -> d1
    u.enable_input(InpSel.ZERO, 3)       # -> d2
    u.enable_input(InpSel.SRC_1, 4)      # v_B -> d3
    u.enable_input(InpSel.SRC_0, 5)      # v_A again -> d4 (z only)
    u.require_inp0 = ENABLE
    u.require_inp1 = ENABLE
    u.trigger = (Trigger.SRC_TENSOR_DONE, Trigger.NONE, Trigger.NONE)
    b = u.datapath_config
    if kind == "z":
        # chain: p=C0-v; q=C1+v; m=min(q,p); z=max(m,0)
        b[0].enable_alu(UAluOp.SUBTRACT, AluInp.PREV_DELAY_0, AluInp.PREV_ALU_OUT)
        b[0].pass_through_delay(0, 1, 2, 3, 4)
        b[1].enable_alu(UAluOp.ADD, AluInp.PREV_DELAY_1, AluInp.PREV_DELAY_4)
        b[1].pass_through_delay(0, 1, 2, 3)
        b[1].enable_delay_from_src(DelayInp.PREV_ALU_OUT, 5)      # p_A
        b[2].enable_alu(UAluOp.MIN, AluInp.PREV_ALU_OUT, AluInp.PREV_DELAY_5)
        b[2].pass_through_delay(0, 1, 2, 3)
        b[3].enable_alu(UAluOp.MAX, AluInp.PREV_ALU_OUT, AluInp.PREV_DELAY_2)
        b[3].pass_through_delay(0, 1, 2, 3)
        b[4].enable_alu(UAluOp.SUBTRACT, AluInp.PREV_DELAY_0, AluInp.PREV_DELAY_3)
        b[4].pass_through_delay(1, 2, 3)
        b[4].enable_delay_from_src(DelayInp.PREV_ALU_OUT, 4)      # z_A
        b[5].enable_alu(UAluOp.ADD, AluInp.PREV_DELAY_1, AluInp.PREV_DELAY_3)
        b[5].pass_through_delay(2, 4)
        b[5].enable_delay_from_src(DelayInp.PREV_ALU_OUT, 5)      # p_B
        b[6].enable_alu(UAluOp.MIN, AluInp.PREV_ALU_OUT, AluInp.PREV_DELAY_5)
        b[6].pass_through_delay(2, 4)
        b[7].enable_alu(UAluOp.MAX, AluInp.PREV_ALU_OUT, AluInp.PREV_DELAY_2)
        b[7].pass_through_delay(4)
    else:
        # chain: r=max(C0-v,0) or max(v+C0,0); out=r*r*r
        if kind == "e5":
            b[0].enable_alu(UAluOp.SUBTRACT, AluInp.PREV_DELAY_0, AluInp.PREV_ALU_OUT)
        else:
            b[0].enable_alu(UAluOp.ADD, AluInp.PREV_ALU_OUT, AluInp.PREV_DELAY_0)
        b[0].pass_through_delay(0, 2, 3)
        b[1].enable_alu(UAluOp.MAX, AluInp.PREV_ALU_OUT, AluInp.PREV_DELAY_2)
        b[1].pass_through_delay(0, 2, 3)
        b[2].enable_alu(UAluOp.MULTIPLY, AluInp.PREV_ALU_OUT, AluInp.PREV_ALU_OUT)
        b[2].pass_through_delay(0, 2, 3)
        b[2].enable_delay_from_src(DelayInp.PREV_ALU_OUT, 4)      # r_A
        b[3].enable_alu(UAluOp.MULTIPLY, AluInp.PREV_ALU_OUT, AluInp.PREV_DELAY_4)
        b[3].pass_through_delay(0, 2, 3)
        if kind == "e5":
            b[4].enable_alu(UAluOp.SUBTRACT, AluInp.PREV_DELAY_0, AluInp.PREV_DELAY_3)
        else:
            b[4].enable_alu(UAluOp.ADD, AluInp.PREV_DELAY_3, AluInp.PREV_DELAY_0)
        b[4].pass_through_delay(2)
        b[4].enable_delay_from_src(DelayInp.PREV_ALU_OUT, 4)      # out_A
        b[5].enable_alu(UAluOp.MAX, AluInp.PREV_ALU_OUT, AluInp.PREV_DELAY_2)
        b[5].pass_through_delay(4)
        b[6].enable_alu(UAluOp.MULTIPLY, AluInp.PREV_ALU_OUT, AluInp.PREV_ALU_OUT)
        b[6].pass_through_delay(4)
        b[6].enable_delay_from_src(DelayInp.PREV_ALU_OUT, 5)      # r_B
        b[7].enable_alu(UAluOp.MULTIPLY, AluInp.PREV_ALU_OUT, AluInp.PREV_DELAY_5)
        b[7].pass_through_delay(4)
    u.out[OutPath.WR0_LO] = OutSel.DELAY_4
    u.out_enable[OutPath.WR0_LO] = ENABLE
    u.out[OutPath.WR1_LO] = OutSel.ALU_OUT
    u.out_enable[OutPath.WR1_LO] = ENABLE
    return u


def _mk_cube_uop(mode):
    """cube = x*x*x per element, fp16 packed operands.
    mode='m1' (2X_1PORT: elems A,B from SRC_0/SRC_0_HI) or
    mode='m4' (4X_2PORT: elems A..D from SRC_0/_HI/SRC_1/_HI)."""
    u = UopConfig()
    u.enable_input(InpSel.SRC_0, 0)       # A -> blk0 ALU
    u.enable_input(InpSel.SRC_0, 1)       # A -> d0
    u.enable_input(InpSel.SRC_0_HI, 2)    # B -> d1
    u.require_inp0 = ENABLE
    if mode == "m4":
        u.enable_input(InpSel.SRC_1, 3)       # C -> d2
        u.enable_input(InpSel.SRC_1_HI, 4)    # D -> d3
        u.require_inp1 = ENABLE
    u.trigger = (Trigger.SRC_TENSOR_DONE, Trigger.NONE, Trigger.NONE)
    b = u.datapath_config
    b[0].enable_alu(UAluOp.MULTIPLY, AluInp.PREV_ALU_OUT, AluInp.PREV_ALU_OUT)
    b[0].pass_through_delay(*((0, 1, 2, 3) if mode == "m4" else (0, 1)))
    b[1].enable_alu(UAluOp.MULTIPLY, AluInp.PREV_ALU_OUT, AluInp.PREV_DELAY_0)
    b[1].pass_through_delay(*((1, 2, 3) if mode == "m4" else (1,)))
    b[2].enable_alu(UAluOp.MULTIPLY, AluInp.PREV_DELAY_1, AluInp.PREV_DELAY_1)
    b[2].pass_through_delay(*((1, 2, 3) if mode == "m4" else (1,)))
    b[2].enable_delay_from_src(DelayInp.PREV_ALU_OUT, 4)       # A^3
    b[3].enable_alu(UAluOp.MULTIPLY, AluInp.PREV_ALU_OUT, AluInp.PREV_DELAY_1)
    b[3].pass_through_delay(*((2, 3, 4) if mode == "m4" else (4,)))
    if mode == "m1":
        b[4].pass_through_delay(4)
        b[4].enable_delay_from_src(DelayInp.PREV_ALU_OUT, 5)   # B^3
        for k in (5, 6, 7):
            b[k].pass_through_delay(4, 5)
        u.out[OutPath.WR0_LO] = OutSel.DELAY_4
        u.out_enable[OutPath.WR0_LO] = ENABLE
        u.out[OutPath.WR0_HI] = OutSel.DELAY_5
        u.out_enable[OutPath.WR0_HI] = ENABLE
        return u
    b[4].enable_alu(UAluOp.MULTIPLY, AluInp.PREV_DELAY_2, AluInp.PREV_DELAY_2)
    b[4].pass_through_delay(2, 3, 4)
    b[4].enable_delay_from_src(DelayInp.PREV_ALU_OUT, 5)       # B^3
    b[5].enable_alu(UAluOp.MULTIPLY, AluInp.PREV_ALU_OUT, AluInp.PREV_DELAY_2)
    b[5].pass_through_delay(3, 4, 5)
    b[6].enable_alu(UAluOp.MULTIPLY, AluInp.PREV_DELAY_3, AluInp.PREV_DELAY_3)
    b[6].pass_through_delay(3, 4, 5)
    b[6].enable_delay_from_src(DelayInp.PREV_ALU_OUT, 0)       # C^3
    b[7].enable_alu(UAluOp.MULTIPLY, AluInp.PREV_ALU_OUT, AluInp.PREV_DELAY_3)
    b[7].pass_through_delay(0, 4, 5)
    u.out[OutPath.WR0_LO] = OutSel.DELAY_4
    u.out_enable[OutPath.WR0_LO] = ENABLE
    u.out[OutPath.WR0_HI] = OutSel.DELAY_5
    u.out_enable[OutPath.WR0_HI] = ENABLE
    u.out[OutPath.WR1_LO] = OutSel.DELAY_0
    u.out_enable[OutPath.WR1_LO] = ENABLE
    u.out[OutPath.WR1_HI] = OutSel.ALU_OUT
    u.out_enable[OutPath.WR1_HI] = ENABLE
    return u


class _PerfDveOp:
    """Duck-typed DveOp whose compiled DveOpSpec carries hand-authored
    perf-mode programs (2X_1PORT filler + 2X_2PORT) with perf_max=2."""

    def __init__(self, name, spec, uop_kind):
        self.name = name
        self.spec = spec
        self.subdim = False
        self.uops_sha = {}
        self._uop_kind = uop_kind
        self._cache = {}

    def compile(self, ver):
        if ver in self._cache:
            return self._cache[ver]
        s = DveOpSpec(
            name=self.name,
            opcode=dve_ops_mod.get_dve_sub_opcode(self.name),
            uops=lower(self.spec, ver=ver),
            rd1_en=_has_src1(self.spec),
        )
        if ver == "v3":
            assert len(s.uops) == 1, (self.name, len(s.uops))
            if self._uop_kind == "cube4x":
                m1 = _mk_cube_uop("m1")
                s.uops_2x = [m1]
                s.uops_2x_2p = [m1]   # unreachable (16-bit srcs go m1/m4)
                s.uops_4x = [_mk_cube_uop("m4")]
                s.perf_max = 3
            else:
                pu = _mk_2x2p_uop(self._uop_kind)
                s.uops_2x = [pu]  # unreachable filler (f32 srcs never pack)
                s.uops_2x_2p = [pu]
                s.perf_max = 2
        self._cache[ver] = s
        return s


def _register_dve_op(name, spec):
    for op in dve_ops_mod.OPS:
        if op.name == name:
            return op
    opcode = dve_ops_mod._CUSTOM_DVE_ROW_BASE + len(dve_ops_mod.OPS)
    assert opcode < 0x20, "custom DVE row overflow"
    shas = {}
    for ver in ("v3", "v4"):
        uops = lower(spec, ver=ver)
        shas[ver] = DveOpSpec(
            name=name, opcode=opcode, uops=uops, rd1_en=_has_src1(spec)
        ).sha(ver)
    op = dve_ops_mod.DveOp(name, spec, subdim=False, uops_sha=shas)
    dve_ops_mod.OPS.append(op)
    dve_ops_mod._SUB_OPCODE_FOR_NAME[name] = opcode
    dve_ops_mod.CUSTOM_DVE_SPECS[name] = spec
    return op


def _register_perf_dve_op(name, spec, uop_kind):
    for op in dve_ops_mod.OPS:
        if op.name == name:
            return op
    opcode = dve_ops_mod._CUSTOM_DVE_ROW_BASE + len(dve_ops_mod.OPS)
    assert opcode < 0x20, "custom DVE row overflow"
    op = _PerfDveOp(name, spec, uop_kind)
    dve_ops_mod.OPS.append(op)
    dve_ops_mod._SUB_OPCODE_FOR_NAME[name] = opcode
    dve_ops_mod.CUSTOM_DVE_SPECS[name] = spec
    op.compile("v3")  # validate lowering + perf programs early
    return op


def _get_z2x_op():
    # out = max(min(s0 - v, s1 + v), 0)   (4 ALU ops; 2 elem/cycle at 2x_2p)
    spec = Spec(
        body=maxx(minn(C0 - Src0, C1 + Src0), Zero),
        reference=lambda in0, in1, s0, s1, imm2: np.maximum(
            np.minimum(s0 - in0, s1 + in0), np.float32(0.0)
        ).astype(np.float32),
    )
    return _register_perf_dve_op("BSPLINE_Z2X_ANT", spec, "z")


def _get_edge5_2x_op():
    # out = cube(max(s0 - v, 0))
    r = maxx(C0 - Src0, Zero)
    spec = Spec(
        body=sq(r) * r,
        reference=lambda in0, in1, s0, s1, imm2: (
            np.maximum(s0 - in0, np.float32(0.0)).astype(np.float32) ** 3
        ).astype(np.float32),
    )
    return _register_perf_dve_op("BSPLINE_E5_2X_ANT", spec, "e5")


def _get_edge12_2x_op():
    # out = cube(max(v + s0, 0))
    r = maxx(Src0 + C0, Zero)
    spec = Spec(
        body=sq(r) * r,
        reference=lambda in0, in1, s0, s1, imm2: (
            np.maximum(in0 + s0, np.float32(0.0)).astype(np.float32) ** 3
        ).astype(np.float32),
    )
    return _register_perf_dve_op("BSPLINE_E12_2X_ANT", spec, "e12")


def _emit_perf_dve(nc, op, *, out, in0, s0=0.0, s1=0.0, perf_max=2):
    bi = nc.vector._custom_dve(op, out=out, in0=in0, s0=s0, s1=s1)
    bi.ins.perf_max = perf_max
    return bi


def _get_cube4x_op():
    # out = in0^3, fp16 packed: 2 elem/cyc at 2X_1PORT, 4 at 4X_2PORT
    spec = Spec(
        body=sq(Src0) * Src0,
        reference=lambda in0, in1, s0, s1, imm2: (
            in0.astype(np.float32) ** 3
        ).astype(np.float32),
    )
    return _register_perf_dve_op("BSPLINE_CUBE4X_ANT", spec, "cube4x")


def _get_cube_diff_op():
    # out = in0^3 - imm2 * relu(in0 - s0)^3        (8 ALU stages)
    r = relu(Src0 - C0)
    body = sq(Src0) * Src0 - sq(r) * r * C2
    spec = Spec(
        body=body,
        reference=lambda in0, in1, s0, s1, imm2: (
            in0.astype(np.float32) ** 3
            - np.maximum(in0 - s0, np.float32(0.0)).astype(np.float32) ** 3 * imm2
        ).astype(np.float32),
    )
    return _register_dve_op("BSPLINE_CUBE_DIFF_ANT", spec)


def _get_z_op():
    # out = relu((2 - |in0*imm2 + s0|) * s1)       (7 ALU stages)
    w = Src0 * C2 + C0
    a = maxx(w, Zero - w)
    body = relu(((One + One) - a) * C1)
    spec = Spec(
        body=body,
        reference=lambda in0, in1, s0, s1, imm2: np.maximum(
            (np.float32(2.0) - np.abs(in0 * imm2 + s0)) * s1, np.float32(0.0)
        ).astype(np.float32),
    )
    return _register_dve_op("BSPLINE_Z_ANT", spec)


def _get_cube_op():
    # out = in0^3                                  (2 ALU stages)
    spec = Spec(
        body=sq(Src0) * Src0,
        reference=lambda in0, in1, s0, s1, imm2: (
            in0.astype(np.float32) ** 3
        ).astype(np.float32),
    )
    return _register_dve_op("BSPLINE_CUBE_ANT", spec)


def _get_edge_cube_op():
    # out = relu(in0*s0 + s1)^3                    (5 ALU stages)
    r = relu(Src0 * C0 + C1)
    spec = Spec(
        body=sq(r) * r,
        reference=lambda in0, in1, s0, s1, imm2: (
            np.maximum(in0 * s0 + s1, np.float32(0.0)).astype(np.float32) ** 3
        ).astype(np.float32),
    )
    return _register_dve_op("BSPLINE_EDGE_CUBE_ANT", spec)


def _register_const(nc, value):
    """Make `value` usable as an activation bias (const_aps lookup).
    Must be called inside the TileContext: the memset is tracked by Tile."""
    f32 = mybir.dt.float32
    key = (f32, float(value))
    if key in nc.const_aps.aps:
        return
    t = nc.alloc_sbuf_tensor(f"const-f32-{float(value)}", [128, 1], f32)
    nc.vector.memset(t.ap(), float(value))
    nc.const_aps.aps[key] = t.ap()


def _build_bass():
    cube_diff_op = _get_cube_diff_op()
    z_op = _get_z_op()
    cube_op = _get_cube_op()
    edge_cube_op = _get_edge_cube_op()
    z2x_op = _get_z2x_op()
    edge5_2x_op = _get_edge5_2x_op()
    edge12_2x_op = _get_edge12_2x_op()
    cube4x_op = _get_cube4x_op()
    f32 = mybir.dt.float32
    # Skip Bass.__init__'s trailing all-engine barrier (only guards its
    # 0.0/1.0 const memsets; the earlier _nrt_pseudo_barrier already orders
    # the semaphore clears).  The only in-kernel reader of those consts is
    # the throwaway table-warm activation below.  Saves ~2us of preamble.
    if SKIP_INIT_BARRIER:
        _orig_barrier = bass.Bass.all_engine_barrier
        bass.Bass.all_engine_barrier = lambda self: None
        try:
            nc = bacc.Bacc(
                "TRN2", target_bir_lowering=False, debug=False,
                num_devices=N_CORES, enable_asserts=ENABLE_ASSERTS,
            )
        finally:
            bass.Bass.all_engine_barrier = _orig_barrier
    else:
        nc = bacc.Bacc(
            "TRN2", target_bir_lowering=False, debug=False,
            num_devices=N_CORES, enable_asserts=ENABLE_ASSERTS,
        )
    f16 = mybir.dt.float16
    x_dt = {np.float16: f16, np.float32: f32}[X_DT_NP]
    x_dram = nc.dram_tensor("x", [N_PAD], x_dt, kind="ExternalInput")
    if BATCH_OUT:
        # chunk-major layout: partition p's row-r chunk-c data lives at
        # [p, N_NZ*lo_c + (r-R_LO)*ch_c + f] -- each chunk's 8 rows are one
        # contiguous run per partition, so one DMA = 128 descriptors.
        out_dram = nc.dram_tensor("out", [P, N_NZ * FD], f16, kind="ExternalOutput")
    else:
        out_dram = nc.dram_tensor("out", [N_NZ, N_PAD], f16, kind="ExternalOutput")
    xv = x_dram.ap().rearrange("(p f) -> p f", p=P)

    with tile.TileContext(nc) as tc:
        with (
            tc.tile_pool(name="const", bufs=1) as cpool,
            tc.tile_pool(name="work", bufs=WBUFS) as wpool,
            tc.tile_pool(name="obuf", bufs=OBUFS) as opool,
            tc.tile_pool(name="psum", bufs=2, space="PSUM") as ppool,
        ):
            zpool = ppool if Z_IN_PSUM else wpool
            x_tile = cpool.tile([P, FD], x_dt, tag="x")
            for ci, (lo, hi) in enumerate(_chunks()):
                if ci == 0 and SPLIT_X0:
                    # halve the first chunk across both HWDGE queues so
                    # compute starts sooner
                    nc.sync.dma_start(out=x_tile[:64, lo:hi], in_=xv[:64, lo:hi])
                    nc.scalar.dma_start(out=x_tile[64:, lo:hi], in_=xv[64:, lo:hi])
                elif ci == 0 and X0_ON_SCALAR:
                    # scalar queue is free ~1us before sync (which is still in
                    # its preamble DRAIN); emitting chunk0 there (before any
                    # ACTIVATE, so ahead of the ACT_TABLE_LOAD) starts the
                    # input pipeline sooner
                    nc.scalar.dma_start(out=x_tile[:, lo:hi], in_=xv[:, lo:hi])
                else:
                    nc.sync.dma_start(out=x_tile[:, lo:hi], in_=xv[:, lo:hi])

            warm = cpool.tile([P, 1], f32, tag="warm")
            nc.scalar.activation(
                warm[:], nc.const_aps.aps[(f32, 0.0)][:P, :],
                mybir.ActivationFunctionType.Abs, bias=0.0, scale=1.0,
            )
            for r in range(R_LO + 1, N_BASIS - 1):
                if r not in V_PATH_RS:
                    _register_const(nc, float(6 - r))
            _register_const(nc, 2.0 * C1V)
            _register_const(nc, C1V)          # bias for edge row 5
            _register_const(nc, -4.0 * C1V)   # bias for edge row 12

            chunks = _chunks()
            for ci, (lo, hi) in enumerate(chunks):
                rows = list(
                    (ROW_ORDER_LAST if ci == len(chunks) - 1 and ROW_ORDER_LAST
                     else ROW_ORDER) or range(R_LO, N_BASIS)
                )
                ch = hi - lo
                xs = x_tile[:, lo:hi]
                batch = BATCH_OUT and ci < len(chunks) - 1
                if batch:
                    o8_t = opool.tile(
                        [P, N_NZ * ch], f16, tag="o8", name=f"o8_{ci}"
                    )
                else:
                    o8_t = None
                need_v = (Z_2X and any(
                    r2 in V_PATH_RS or (r2, ci) in V_PATH_EXTRA
                    for r2 in range(R_LO + 1, N_BASIS - 1)
                )) or EDGE_2X
                if need_v:
                    # v = 5c*x, shared by the 2x z/edge ops of this chunk
                    v_t = wpool.tile([P, ch], f32, tag="v")
                    if V_ON_SCALAR:
                        nc.scalar.activation(
                            v_t[:], xs, mybir.ActivationFunctionType.Copy,
                            bias=0.0, scale=5.0 * C1V,
                        )
                    else:
                        eng = nc.gpsimd if V_ON_GPSIMD else nc.vector
                        eng.tensor_scalar_mul(v_t[:], xs, 5.0 * C1V)
                for r in rows:
                    on_v = r in V_PATH_RS or (r, ci) in V_PATH_EXTRA
                    if batch:
                        ri = r - R_LO
                        o_ap = o8_t[:, ri * ch : (ri + 1) * ch]
                    else:
                        o_t = wpool.tile([P, ch], f16, tag="o")
                        o_ap = o_t[:]
                    if r == R_LO and EDGE_2X:
                        # out_5 = cube(max(c - v, 0))  -- 2 elem/cycle
                        _emit_perf_dve(
                            nc, edge5_2x_op, out=o_ap, in0=v_t[:], s0=C1V,
                        )
                    elif r == N_BASIS - 1 and E12_4X:
                        # out_12: ScalarE relu (fp16) + 4-elem/cycle cube
                        et = wpool.tile([P, ch], f16, tag="a")
                        nc.scalar.activation(
                            et[:], xs, mybir.ActivationFunctionType.Relu,
                            bias=-4.0 * C1V, scale=5.0 * C1V,
                        )
                        _emit_perf_dve(
                            nc, cube4x_op, out=o_ap, in0=et[:], perf_max=3,
                        )
                    elif r == N_BASIS - 1 and EDGE_2X:
                        # out_12 = cube(max(v - 4c, 0))  -- 2 elem/cycle
                        _emit_perf_dve(
                            nc, edge12_2x_op, out=o_ap, in0=v_t[:],
                            s0=-4.0 * C1V,
                        )
                    elif r == R_LO and EDGE_ON_V:
                        # out_5 = cube(relu(-5c*x + c))  -- one DVE op
                        nc.vector._custom_dve(
                            edge_cube_op, out=o_ap, in0=xs,
                            s0=-5.0 * C1V, s1=C1V,
                        )
                    elif r == N_BASIS - 1 and EDGE_ON_V:
                        # out_12 = cube(relu(5c*x - 4c))  -- one DVE op
                        nc.vector._custom_dve(
                            edge_cube_op, out=o_ap, in0=xs,
                            s0=5.0 * C1V, s1=-4.0 * C1V,
                        )
                    elif r == R_LO:
                        # out_5 = cube(relu(c*(1 - 5x)))
                        z_t = wpool.tile([P, hi - lo], f32, tag="z")
                        nc.scalar.activation(
                            z_t[:], xs, mybir.ActivationFunctionType.Relu,
                            bias=C1V, scale=-5.0 * C1V,
                        )
                        nc.vector._custom_dve(cube_op, out=o_ap, in0=z_t[:])
                    elif r == N_BASIS - 1:
                        # out_12 = cube(relu(c*(5x - 4)))
                        z_t = wpool.tile([P, hi - lo], f32, tag="z")
                        nc.scalar.activation(
                            z_t[:], xs, mybir.ActivationFunctionType.Relu,
                            bias=-4.0 * C1V, scale=5.0 * C1V,
                        )
                        nc.vector._custom_dve(cube_op, out=o_ap, in0=z_t[:])
                    else:
                        z_t = (wpool if on_v else zpool).tile(
                            [P, hi - lo], f32, tag="z"
                        )
                        if on_v and Z_2X:
                            # z = max(min(c(r-4) - v, c(8-r) + v), 0)
                            _emit_perf_dve(
                                nc, z2x_op, out=z_t[:], in0=v_t[:],
                                s0=(r - 4) * C1V, s1=(8 - r) * C1V,
                            )
                        elif on_v:
                            # z = relu((2 - |5x + (6-r)|) * c)   -- one DVE op
                            nc.vector._custom_dve(
                                z_op, out=z_t[:], in0=xs,
                                s0=float(6 - r), s1=C1V, imm2=5.0,
                            )
                        else:
                            # a = |5x + (6-r)|; z = relu(-c*a + 2c) -- ScalarE
                            a_t = wpool.tile([P, hi - lo], f32, tag="a")
                            nc.scalar.activation(
                                a_t[:], xs, mybir.ActivationFunctionType.Abs,
                                bias=float(6 - r), scale=5.0,
                            )
                            nc.scalar.activation(
                                z_t[:], a_t[:],
                                mybir.ActivationFunctionType.Relu,
                                bias=2.0 * C1V, scale=-C1V,
                            )
                        split = (
                            SPLIT_LAST_ROW and not batch and r == rows[-1]
                            and BATCH_OUT
                        )
                        if split:
                            # final row: two half-width ops + DMAs so the
                            # first half's writeback overlaps the second
                            # half's compute (shorter drain tail)
                            h = int(ch * SPLIT_FRAC) & ~1
                            base = N_NZ * lo + (r - R_LO) * ch
                            for a, b2 in ((0, h), (h, ch)):
                                nc.vector._custom_dve(
                                    cube_diff_op, out=o_ap[:, a:b2],
                                    in0=z_t[:, a:b2], s0=C1V, imm2=4.0,
                                )
                                nc.sync.dma_start(
                                    out=out_dram.ap()[:, base + a : base + b2],
                                    in_=o_ap[:, a:b2],
                                )
                        else:
                            # out = z^3 - 4*relu(z - c)^3
                            nc.vector._custom_dve(
                                cube_diff_op, out=o_ap, in0=z_t[:],
                                s0=C1V, imm2=4.0,
                            )
                    if r in (R_LO, N_BASIS - 1):
                        split = False
                    if not batch and not split:
                        if BATCH_OUT:
                            ov = out_dram.ap()[
                                :, N_NZ * lo + (r - R_LO) * ch :
                                N_NZ * lo + (r - R_LO + 1) * ch
                            ]
                        else:
                            ov = out_dram.ap()[r - R_LO, :].rearrange(
                                "(p f) -> p f", p=P
                            )[:, lo:hi]
                        nc.sync.dma_start(out=ov, in_=o_ap)
                if batch:
                    ov = out_dram.ap()[:, N_NZ * lo : N_NZ * hi]
                    nc.sync.dma_start(out=ov, in_=o8_t[:])
    nc.compile()
    return nc


def make_shard(x, i):
    """Build core i's padded input shard in the device input dtype."""
    sh = np.full(N_PAD, X_PAD_VAL, dtype=X_DT_NP)
    sh[:N_ELEM] = np.ascontiguousarray(
        x[:, i * N_SHARD : (i + 1) * N_SHARD]
    ).reshape(-1).astype(X_DT_NP)
    return sh


_NC_CACHE = None


def _get_nc():
    global _NC_CACHE
    if _NC_CACHE is None:
        _NC_CACHE = _build_bass()
    return _NC_CACHE


def kernel(x, grid=None, k=None, **_ignored):
    x = np.asarray(x, dtype=np.float32)
    assert x.shape == (N_ROWS, N_FULL), x.shape
    nc = _get_nc()
    in_maps = [{"x": make_shard(x, i)} for i in range(N_CORES)]
    res = run_bass_kernel_spmd(nc, in_maps, list(range(N_CORES))).results
    full = np.zeros((N_ROWS, N_BASIS, N_FULL), dtype=np.float32)
    for i in range(N_CORES):
        o = np.asarray(res[i]["out"])
        if BATCH_OUT:
            # [P, N_NZ*FD] chunk-major -> [N_NZ, P, FD] -> [N_NZ, N_PAD]
            rows = np.empty((N_NZ, P, FD), dtype=np.float32)
            for lo, hi in _chunks():
                seg = o[:, N_NZ * lo : N_NZ * hi].reshape(P, N_NZ, hi - lo)
                rows[:, :, lo:hi] = seg.transpose(1, 0, 2)
            o = rows.reshape(N_NZ, N_PAD)
        else:
            o = o.astype(np.float32)  # [N_NZ, N_PAD]
        full[:, R_LO:, i * N_SHARD : (i + 1) * N_SHARD] = o[:, :N_ELEM].reshape(
            N_NZ, N_ROWS, N_SHARD
        ).transpose(1, 0, 2)
    return full

